# revision 19
# baseline (speedup 1.0000x reference)
"""Trainium2 Bass kernel for nn_KohaNetwork.

Strategy (context/block parallelism, 8 blocks per core):
  - Host folds Wq@Wk^T into one matrix M per block, so the device never
    computes k:  s[b,r] = z[b,r,:] . (M^T x[b,:])  with M^T = Wq Wk^T.
  - Device per block: gT = MT^T @ xT (PE), v = z @ Wv (PE, fp32r),
    S = zT^T @ gT (PE), softmax via masked reduce + PE mask-matmul sums,
    y = A_un^T @ v (PE, fp32r), tanh(y * 1/sums) (ACT), DMA out.
  - Host scatters per-block outputs back into the state copy.
"""

import os
import sys

import numpy as np

for _p in ("/opt/trn_rl_repo", "/root/.axon_site/_ro/trn_rl_repo"):
    if os.path.isdir(_p) and _p not in sys.path:
        sys.path.insert(0, _p)

import concourse.bass as bass
import concourse.bacc as bacc
import concourse.mybir as mybir
import concourse.tile as tile
from concourse.bass_utils import run_bass_kernel_spmd

EMB = 512
C = 64
RF = 16
B = 32
L = 79
NCORES = 8
JPC = C // NCORES  # blocks per core = 8

DT = mybir.dt.float32
DTR = mybir.dt.float32r
USE_FP32R = True
INV_SQRT_D = 1.0 / float(np.sqrt(np.float32(EMB)))

_PROG = None
last_results = None


# dtype used for operands of the big (N=512) matmuls; float32r runs the PE
# at 1 cycle/row instead of 4.
DT_R = DTR if USE_FP32R else DT


def build_program():
    nc = bacc.Bacc(None, target_bir_lowering=False)
    wm = nc.declare_dram_parameter("wm", [JPC, EMB, EMB], DT, isOutput=False)
    wv = nc.declare_dram_parameter("wv", [JPC, EMB, EMB], DT_R, isOutput=False)
    xt = nc.declare_dram_parameter("xt", [JPC, EMB, B], DT, isOutput=False)
    st = nc.declare_dram_parameter("st", [EMB, B, JPC + RF - 1], DT, isOutput=False)
    mask = nc.declare_dram_parameter("mask", [128, 4, B], DT, isOutput=False)
    y = nc.declare_dram_parameter("y", [JPC, B, EMB], DT, isOutput=True)

    Exp = mybir.ActivationFunctionType.Exp
    Tanh = mybir.ActivationFunctionType.Tanh

    from contextlib import ExitStack

    with tile.TileContext(nc) as tc, ExitStack() as ctx:
        const = ctx.enter_context(tc.tile_pool(name="const", bufs=1))
        wpool = ctx.enter_context(tc.tile_pool(name="wpool", bufs=4))
        work = ctx.enter_context(tc.tile_pool(name="work", bufs=8))
        psb = ctx.enter_context(tc.tile_pool(name="psb", bufs=3, space="PSUM"))
        pss = ctx.enter_context(tc.tile_pool(name="pss", bufs=4, space="PSUM"))
        psyp = ctx.enter_context(tc.tile_pool(name="psyp", bufs=1, space="PSUM"))

        # ---- persistent tiles (order tuned: block 0's operands first) ----
        xt_sb = []
        for u in range(4):
            t_xt = const.tile([128, JPC, B], DT, tag=f"xt{u}", name=f"xt{u}")
            nc.sync.dma_start(
                t_xt[:], xt[:, u * 128:(u + 1) * 128, :].rearrange("j d b -> d j b")
            )
            xt_sb.append(t_xt)
        st_sb = [None] * 4

        def load_st():
            for u in range(4):
                t_st = const.tile(
                    [128, B, JPC + RF - 1], DT, tag=f"st{u}", name=f"st{u}"
                )
                nc.sync.dma_start(t_st[:], st[u * 128:(u + 1) * 128, :, :])
                st_sb[u] = t_st

        mask_sb = const.tile([128, 4, B], DT, tag="mask", name="mask")
        nc.sync.dma_start(mask_sb[:], mask[:])

        # Two zT ring buffers (even/odd blocks) per d-tile.  Ring slot of
        # state column c is c % RF; the r-order inside each b-group is a
        # permutation, which S/softmax/y are invariant to (y sums over rows,
        # and A_un rows come from the same ring as v rows).
        ztr = [[const.tile([128, B * RF], DT_R, tag=f"zt{p}_{u}", name=f"zt{p}_{u}")
                for u in range(4)] for p in range(2)]
        ztf = [[const.tile([128, B * RF], DT, tag=f"ztf{p}_{u}", name=f"ztf{p}_{u}")
                for u in range(4)] for p in range(2)]

        state = {}

        def ring_update(jj):
            """Make ring jj%2 hold the window for block jj."""
            par = jj % 2
            for u in range(4):
              for ring in (ztr, ztf):
                zt3 = ring[par][u][:].rearrange("p (b r) -> p b r", r=RF)
                if jj == 0:
                    nc.vector.tensor_copy(zt3, st_sb[u][:, :, 0:RF])
                elif jj == 1:
                    # cols 1..16 -> slots 1..15, 0
                    nc.vector.tensor_copy(
                        zt3[:, :, 1:RF], st_sb[u][:, :, 1:RF]
                    )
                    nc.vector.tensor_copy(
                        zt3[:, :, 0:1], st_sb[u][:, :, RF:RF + 1]
                    )
                else:
                    # replace slots (jj-2)%RF,(jj-1)%RF with cols jj+14, jj+15
                    s0 = (jj - 2) % RF
                    nc.vector.tensor_copy(
                        zt3[:, :, s0:s0 + 2],
                        st_sb[u][:, :, jj + RF - 2:jj + RF],
                    )

        wtiles = {}

        def prefetch(jj):
            wm_sb = wpool.tile([128, 4, EMB], DT, tag="wm", name=f"wm{jj}")
            nc.sync.dma_start(
                wm_sb[:], wm[jj].rearrange("(t p) e -> p t e", p=128)
            )
            wv_sb = wpool.tile([128, 4, EMB], DT_R, tag="wv", name=f"wv{jj}")
            nc.sync.dma_start(
                wv_sb[:], wv[jj].rearrange("(t p) e -> p t e", p=128)
            )
            wtiles[jj] = (wm_sb, wv_sb)

        def stage1(jj):
            if jj not in wtiles:
                prefetch(jj)
            wm_sb, wv_sb = wtiles.pop(jj)
            zt_sb = ztr[jj % 2]
            ztf_sb = ztf[jj % 2]

            # gT[d, b] = sum_d' MT[d', d] * xT[d', b] — all 4 d-chunks into
            # one PSUM tile, drained with a single DVE copy.
            psg = pss.tile([128, 4, B], DT, tag="ps_small", name=f"psg{jj}")
            for t in range(4):
                for u in range(4):
                    nc.tensor.matmul(
                        psg[:, t, :],
                        wm_sb[:, u, t * 128:(t + 1) * 128],
                        xt_sb[u][:, jj, :],
                        start=(u == 0),
                        stop=(u == 3),
                    )
            g_sb = work.tile([128, 4, B], DT, tag="gsb", name=f"g{jj}")
            nc.vector.tensor_copy(g_sb[:], psg[:])

            # S[(br), b'] = sum_d zT[d, br] * gT[d, b'] — 4 br-chunks into one
            # PSUM tile; single exp / mask-mul / reduce over [128, 128].
            psS = pss.tile([128, 4, B], DT, tag="ps_small", name=f"psS{jj}")
            for t in range(4):
                for u in range(4):
                    nc.tensor.matmul(
                        psS[:, t, :],
                        ztf_sb[u][:, 128 * t:128 * (t + 1)],
                        g_sb[:, u, :],
                        start=(u == 0),
                        stop=(u == 3),
                    )
            expf = work.tile([128, 4, B], DT, tag="expf", name=f"expf{jj}")
            nc.scalar.activation(expf[:], psS[:], Exp, scale=INV_SQRT_D)
            aun = work.tile([128, 4, B], DT_R, tag="aun", name=f"aun{jj}")
            nc.vector.tensor_tensor(
                aun[:], expf[:], mask_sb[:], mybir.AluOpType.mult
            )
            exps = work.tile([128, 4], DT, tag="exps", name=f"exps{jj}")
            nc.vector.tensor_reduce(
                exps[:], aun[:], mybir.AxisListType.X, mybir.AluOpType.add
            )

            # v[(br), e] = sum_d z[br, d] * Wv[d, e]   (4 br-tiles)
            v_sb = []
            for t in range(4):
                psv = psb.tile([128, EMB], DT, tag="psv", name=f"psv{jj}_{t}")
                for u in range(4):
                    nc.tensor.matmul(
                        psv[:],
                        zt_sb[u][:, 128 * t:128 * (t + 1)],
                        wv_sb[:, u, :],
                        start=(u == 0),
                        stop=(u == 3),
                    )
                v_t = work.tile([128, EMB], DT_R, tag="vsb", name=f"v{jj}_{t}")
                if t % 2 == 0:
                    nc.vector.tensor_copy(v_t[:], psv[:])
                else:
                    nc.scalar.copy(v_t[:], psv[:])
                v_sb.append(v_t)

            state[jj] = (v_sb, aun, exps)

        def stage2(jj):
            v_sb, aun, exps = state.pop(jj)
            pssum = pss.tile([B, 1], DT, tag="ps_small", name=f"pssum{jj}")
            for t in range(4):
                nc.tensor.matmul(
                    pssum[:],
                    mask_sb[:, t, :],
                    exps[:, t:t + 1],
                    start=(t == 0),
                    stop=(t == 3),
                )
            inv = work.tile([B, 1], DT, tag="inv", name=f"inv{jj}")
            nc.vector.reciprocal(inv[:], pssum[:])

            psy = psyp.tile([B, EMB], DT, tag="psy", name=f"psy{jj}")
            for t in range(4):
                nc.tensor.matmul(
                    psy[:],
                    aun[:, t, :],
                    v_sb[t][:],
                    start=(t == 0),
                    stop=(t == 3),
                )
            yo = work.tile([B, EMB], DT, tag="yo", bufs=4, name=f"yo{jj}")
            nc.scalar.activation(yo[:], psy[:], Tanh, scale=inv[:])
            nc.sync.dma_start(y[jj], yo[:])

        prefetch(0)
        load_st()
        prefetch(1)
        ring_update(0)
        ring_update(1)
        for jj in range(JPC):
            stage1(jj)
            if jj + 2 < JPC:
                prefetch(jj + 2)
                ring_update(jj + 2)
            if jj >= 1:
                stage2(jj - 1)
        stage2(JPC - 1)

    nc.compile()
    return nc


def _host_prep(input_indices, network_state, emb_table, Wq, Wk, Wv):
    idx = np.asarray(input_indices)
    state = np.asarray(network_state, dtype=np.float32)
    emb = np.asarray(emb_table, dtype=np.float32)
    Wq = np.asarray(Wq, dtype=np.float32)
    Wk = np.asarray(Wk, dtype=np.float32)
    Wv = np.asarray(Wv, dtype=np.float32)

    inp = emb[idx[:, 0]]                                   # [B, D]
    stT = np.ascontiguousarray(state.transpose(1, 0, 2))   # [D, B, L]
    XT = np.empty((C, EMB, B), np.float32)
    XT[0] = inp.T
    XT[1:] = stT[:, :, 0:C - 1].transpose(2, 0, 1)
    MT = np.matmul(Wq, Wk.transpose(0, 2, 1))              # [C, D', D]

    mask = np.zeros((128, 4, B), np.float32)
    p = np.arange(128)
    for t in range(4):
        mask[p, t, 8 * t + p // 16] = 1.0

    in_maps = []
    for c in range(NCORES):
        j0 = JPC * c
        in_maps.append({
            "wm": np.ascontiguousarray(MT[j0:j0 + JPC]),
            "wv": np.ascontiguousarray(Wv[j0:j0 + JPC]),
            "xt": np.ascontiguousarray(XT[j0:j0 + JPC]),
            "st": np.ascontiguousarray(stT[:, :, j0:j0 + JPC + RF - 1]),
            "mask": mask,
        })
    return state, in_maps


def kernel(input_indices, network_state, emb_table, Wq, Wk, Wv):
    global _PROG, last_results
    state, in_maps = _host_prep(
        input_indices, network_state, emb_table, Wq, Wk, Wv
    )
    if _PROG is None:
        _PROG = build_program()
    last_results = run_bass_kernel_spmd(
        _PROG, in_maps, core_ids=list(range(NCORES))
    )
    new_state = state.copy()
    for c in range(NCORES):
        yc = np.asarray(last_results.results[c]["y"])      # [JPC, B, EMB]
        new_state[:, :, JPC * c:JPC * (c + 1)] = yc.transpose(1, 2, 0)
    return new_state

